# revision 1
# baseline (speedup 1.0000x reference)
import sys
if '/opt/trn_rl_repo' not in sys.path:
    sys.path.insert(0, '/opt/trn_rl_repo')
import numpy as np
import ml_dtypes

import concourse.bass as bass
import concourse.bacc as bacc
import concourse.mybir as mybir
import concourse.tile as tile
from concourse.masks import make_identity

P = 128
N_CORES = 8
LAYERS = 2
RG = [list(range(N_CORES))]

F32 = mybir.dt.float32
BF16 = mybir.dt.bfloat16
I32 = mybir.dt.int32
NP_BF16 = ml_dtypes.bfloat16


def _cdiv(a, b):
    return -(-a // b)


# ---------------------------------------------------------------------------
# host-side edge packing
# ---------------------------------------------------------------------------

def pack_edges(rows, cols, vals, nblk_core, ch, edge_np_dt):
    """Sort edges by row, bucket into 128-row blocks, pad each block to
    ch chunks of 128 edges. Returns per-core packed arrays:
      idx  [N_CORES, 128, nblk_core*ch] int32   (column index per edge slot)
      lr   [N_CORES, 128, nblk_core*ch] edge_dt (row % 128 per edge slot)
      val  [N_CORES, 128, nblk_core*ch] edge_dt (edge value, 0 for padding)
    Slot (core, lane p, b*ch + j) holds edge #(j*128+p) of local block b.
    """
    nblk_total = N_CORES * nblk_core
    order = np.argsort(rows, kind='stable')
    r = rows[order].astype(np.int64)
    c = cols[order].astype(np.int64)
    v = vals[order]
    blk = r >> 7
    counts = np.bincount(blk, minlength=nblk_total)
    offs = np.zeros(nblk_total + 1, np.int64)
    np.cumsum(counts, out=offs[1:])
    rank = np.arange(len(r), dtype=np.int64) - offs[blk]
    j = rank >> 7
    lane = rank & 127
    core = blk // nblk_core
    lblk = blk % nblk_core
    colpos = lblk * ch + j
    W = nblk_core * ch
    idx = np.zeros((N_CORES, P, W), np.int32)
    lr = np.zeros((N_CORES, P, W), np.float32)
    val = np.zeros((N_CORES, P, W), np.float32)
    idx[core, lane, colpos] = c
    lr[core, lane, colpos] = (r & 127)
    val[core, lane, colpos] = v
    return idx, lr.astype(edge_np_dt), val.astype(edge_np_dt)


def max_chunks(rows, nblk_core):
    counts = np.bincount(np.asarray(rows, np.int64) >> 7,
                         minlength=N_CORES * nblk_core)
    return max(1, int(_cdiv(int(counts.max()), P)))


# ---------------------------------------------------------------------------
# bass program builder
# ---------------------------------------------------------------------------

def build_program(cfg):
    """Build the SPMD bass program for one core (identical across cores)."""
    NU = cfg['NU']; NS = cfg['NS']; NM = cfg['NM']; D = cfg['D']
    NBU = cfg['NBU']            # ui blocks per core
    NBM = cfg['NBM']            # mashup blocks per core
    CHU = cfg['CHU']            # chunks per ui block
    CHM = cfg['CHM']            # chunks per mashup block
    BT = cfg['BT']              # batch tiles per core (of 128)
    L = cfg['L']
    EDGE_DT = BF16 if cfg['bf16'] else F32
    GG = cfg['GG']              # blocks per gather group

    RWU = NBU * P               # ui rows per core
    RWM = NBM * P
    UI_ROWS = N_CORES * RWU     # padded full table sizes
    MM_ROWS = N_CORES * RWM

    nc = bacc.Bacc("TRN2", target_bir_lowering=False, debug=False,
                   enable_asserts=False, num_devices=N_CORES)

    def din(name, shape, dt):
        return nc.dram_tensor(name, shape, dt, kind="ExternalInput").ap()

    x0s = din("x0s", [RWU, D], F32)            # this core's ui row shard (padded)
    m0s = din("m0s", [RWM, D], F32)            # this core's mashup row shard
    ui_idx = din("ui_idx", [P, NBU * CHU], I32)
    ui_lr = din("ui_lr", [P, NBU * CHU], EDGE_DT)
    ui_val = din("ui_val", [P, NBU * CHU], EDGE_DT)
    mm_idx = din("mm_idx", [P, NBM * CHM], I32)
    mm_lr = din("mm_lr", [P, NBM * CHM], EDGE_DT)
    mm_val = din("mm_val", [P, NBM * CHM], EDGE_DT)
    dinv = din("dinv", [P, NBM], F32)          # d_inv packed (row%128, block)
    memb_idx = din("memb_idx", [BT * P, L], I32)
    svc_idx = din("svc_idx", [BT * P, 1], I32)
    mash_idx = din("mash_idx", [BT * P, 1], I32)
    mask_lb = din("mask_lb", [BT, L * P], F32)  # (t, (l, b)) layout, 1=pad
    aw1 = din("aw1", [2 * D, 16], F32)
    ab1 = din("ab1", [16], F32)
    aw2 = din("aw2", [16, 1], F32)
    ab2 = din("ab2", [1], F32)
    pw1 = din("pw1", [3 * D, 8], F32)
    pb1 = din("pb1", [8], F32)
    pw2 = din("pw2", [8, 1], F32)
    pb2 = din("pb2", [1], F32)

    y = nc.dram_tensor("y", [BT * P, 1], F32, kind="ExternalOutput").ap()

    with tile.TileContext(nc) as tc:
        with tc.tile_pool(name="dram", bufs=1, space="DRAM") as dram, \
             tc.tile_pool(name="res", bufs=1) as res:
            # DRAM bounce buffers for collectives
            x0bf_in = dram.tile([RWU, D], EDGE_DT)
            x0bf_full = dram.tile([UI_ROWS, D], EDGE_DT, addr_space="Shared")
            m0bf_in = dram.tile([RWM, D], EDGE_DT)
            m0bf_full = dram.tile([MM_ROWS, D], EDGE_DT, addr_space="Shared")
            h1bf_in = dram.tile([RWU, D], EDGE_DT)
            h1bf_full = dram.tile([UI_ROWS, D], EDGE_DT, addr_space="Shared")
            m1bf_in = dram.tile([RWM, D], EDGE_DT)
            m1bf_full = dram.tile([MM_ROWS, D], EDGE_DT, addr_space="Shared")
            uif_in = dram.tile([RWU, D], F32)
            uif_full = dram.tile([UI_ROWS, D], F32, addr_space="Shared")
            macc_in = dram.tile([RWM, D], F32)
            macc_full = dram.tile([MM_ROWS, D], F32, addr_space="Shared")

            # resident SBUF
            acc_u = res.tile([P, NBU * D], F32)       # x0 + h1 + h2 (row shard)
            acc_m = res.tile([P, NBM * D], F32)
            iota_t = res.tile([P, D], EDGE_DT)
            nc.gpsimd.iota(iota_t[:], [[1, D]], base=0, channel_multiplier=0,
                           allow_small_or_imprecise_dtypes=True)
            uiidx_t = res.tile([P, NBU * CHU], I32)
            nc.sync.dma_start(out=uiidx_t[:], in_=ui_idx[:])
            uilr_t = res.tile([P, NBU * CHU], EDGE_DT)
            nc.sync.dma_start(out=uilr_t[:], in_=ui_lr[:])
            uival_t = res.tile([P, NBU * CHU], EDGE_DT)
            nc.sync.dma_start(out=uival_t[:], in_=ui_val[:])
            mmidx_t = res.tile([P, NBM * CHM], I32)
            nc.sync.dma_start(out=mmidx_t[:], in_=mm_idx[:])
            mmlr_t = res.tile([P, NBM * CHM], EDGE_DT)
            nc.sync.dma_start(out=mmlr_t[:], in_=mm_lr[:])
            mmval_t = res.tile([P, NBM * CHM], EDGE_DT)
            nc.sync.dma_start(out=mmval_t[:], in_=mm_val[:])
            dinv_t = res.tile([P, NBM], F32)
            nc.sync.dma_start(out=dinv_t[:], in_=dinv[:])

            # load shards into acc (one big strided DMA each)
            nc.sync.dma_start(
                out=acc_u[:].rearrange("p (b d) -> p b d", d=D),
                in_=x0s.rearrange("(b p) d -> p b d", p=P))
            nc.sync.dma_start(
                out=acc_m[:].rearrange("p (b d) -> p b d", d=D),
                in_=m0s.rearrange("(b p) d -> p b d", p=P))

            # ----------------- phase A: bf16 convert + AG base tables -------
            with tc.tile_pool(name="conv", bufs=3) as cv:
                grp = 7
                for g in range(_cdiv(NBU, grp)):
                    b0 = g * grp
                    nb = min(grp, NBU - b0)
                    cvt = cv.tile([P, grp * D], EDGE_DT, tag="cvt")
                    nc.vector.tensor_copy(
                        out=cvt[:, :nb * D],
                        in_=acc_u[:, b0 * D:(b0 + nb) * D])
                    nc.sync.dma_start(
                        out=x0bf_in[b0 * P:(b0 + nb) * P, :]
                            .rearrange("(b p) d -> p b d", p=P),
                        in_=cvt[:, :nb * D].rearrange("p (b d) -> p b d", d=D))
                for g in range(_cdiv(NBM, grp)):
                    b0 = g * grp
                    nb = min(grp, NBM - b0)
                    cvt = cv.tile([P, grp * D], EDGE_DT, tag="cvt")
                    nc.vector.tensor_copy(
                        out=cvt[:, :nb * D],
                        in_=acc_m[:, b0 * D:(b0 + nb) * D])
                    nc.sync.dma_start(
                        out=m0bf_in[b0 * P:(b0 + nb) * P, :]
                            .rearrange("(b p) d -> p b d", p=P),
                        in_=cvt[:, :nb * D].rearrange("p (b d) -> p b d", d=D))
            nc.gpsimd.collective_compute(
                "AllGather", mybir.AluOpType.bypass, replica_groups=RG,
                ins=[x0bf_in[:]], outs=[x0bf_full[:]])
            nc.gpsimd.collective_compute(
                "AllGather", mybir.AluOpType.bypass, replica_groups=RG,
                ins=[m0bf_in[:]], outs=[m0bf_full[:]])

            # ----------------- spmm layer helper ----------------------------
            def spmm_layer(src_tbl, nblk, ch, idx_t, lr_t, val_t, post):
                with tc.tile_pool(name="sp_sb", bufs=3) as sp, \
                     tc.tile_pool(name="sp_ps", bufs=6, space="PSUM") as pp:
                    for g0 in range(0, nblk, GG):
                        ng = min(GG, nblk - g0)
                        gt = sp.tile([P, GG * ch * D], EDGE_DT, tag="gt")
                        nc.gpsimd.indirect_dma_start(
                            out=gt[:, :ng * ch * D],
                            out_offset=None,
                            in_=src_tbl[:],
                            in_offset=bass.IndirectOffsetOnAxis(
                                ap=idx_t[:, g0 * ch:(g0 + ng) * ch], axis=0))
                        st = sp.tile([P, GG * ch * D], EDGE_DT, tag="st")
                        s3 = st[:, :ng * ch * D].rearrange("p (k d) -> p k d", d=D)
                        nc.vector.tensor_tensor(
                            out=s3,
                            in0=iota_t[:].unsqueeze(1).to_broadcast([P, ng * ch, D]),
                            in1=lr_t[:, g0 * ch:(g0 + ng) * ch]
                                .to_broadcast([P, ng * ch, D]),
                            op=mybir.AluOpType.is_equal)
                        nc.vector.tensor_tensor(
                            out=s3, in0=s3,
                            in1=val_t[:, g0 * ch:(g0 + ng) * ch]
                                .to_broadcast([P, ng * ch, D]),
                            op=mybir.AluOpType.mult)
                        for bb in range(ng):
                            b = g0 + bb
                            pm = pp.tile([P, D], F32, tag="pm")
                            for j in range(ch):
                                o = (bb * ch + j) * D
                                nc.tensor.matmul(
                                    out=pm[:], lhsT=st[:, o:o + D],
                                    rhs=gt[:, o:o + D],
                                    start=(j == 0), stop=(j == ch - 1))
                            post(sp, b, pm)

            # ----------------- UI layer 1 -----------------------------------
            def ui_l1_post(sp, b, pm):
                h1t = sp.tile([P, D], EDGE_DT, tag="h1t")
                nc.vector.tensor_copy(out=h1t[:], in_=pm[:])
                nc.sync.dma_start(out=h1bf_in[b * P:(b + 1) * P, :], in_=h1t[:])
                nc.vector.tensor_tensor(
                    out=acc_u[:, b * D:(b + 1) * D],
                    in0=acc_u[:, b * D:(b + 1) * D], in1=pm[:],
                    op=mybir.AluOpType.add)

            spmm_layer(x0bf_full, NBU, CHU, uiidx_t, uilr_t, uival_t, ui_l1_post)
            nc.gpsimd.collective_compute(
                "AllGather", mybir.AluOpType.bypass, replica_groups=RG,
                ins=[h1bf_in[:]], outs=[h1bf_full[:]])

            # ----------------- MM layer 1 -----------------------------------
            def mm_l1_post(sp, b, pm):
                tmp = sp.tile([P, D], F32, tag="mtmp")
                nc.vector.tensor_scalar(
                    out=tmp[:], in0=pm[:], scalar1=dinv_t[:, b:b + 1],
                    scalar2=None, op0=mybir.AluOpType.mult)
                m1t = sp.tile([P, D], EDGE_DT, tag="m1t")
                nc.vector.tensor_copy(out=m1t[:], in_=tmp[:])
                nc.sync.dma_start(out=m1bf_in[b * P:(b + 1) * P, :], in_=m1t[:])
                nc.vector.tensor_tensor(
                    out=acc_m[:, b * D:(b + 1) * D],
                    in0=acc_m[:, b * D:(b + 1) * D], in1=tmp[:],
                    op=mybir.AluOpType.add)

            spmm_layer(m0bf_full, NBM, CHM, mmidx_t, mmlr_t, mmval_t, mm_l1_post)
            nc.gpsimd.collective_compute(
                "AllGather", mybir.AluOpType.bypass, replica_groups=RG,
                ins=[m1bf_in[:]], outs=[m1bf_full[:]])

            # ----------------- UI layer 2 -----------------------------------
            def ui_l2_post(sp, b, pm):
                uft = sp.tile([P, D], F32, tag="uft")
                nc.vector.tensor_tensor(
                    out=uft[:], in0=acc_u[:, b * D:(b + 1) * D], in1=pm[:],
                    op=mybir.AluOpType.add)
                nc.scalar.mul(uft[:], uft[:], 1.0 / (LAYERS + 1))
                nc.sync.dma_start(out=uif_in[b * P:(b + 1) * P, :], in_=uft[:])

            spmm_layer(h1bf_full, NBU, CHU, uiidx_t, uilr_t, uival_t, ui_l2_post)
            nc.gpsimd.collective_compute(
                "AllGather", mybir.AluOpType.bypass, replica_groups=RG,
                ins=[uif_in[:]], outs=[uif_full[:]])

            # ----------------- MM layer 2 -----------------------------------
            def mm_l2_post(sp, b, pm):
                mct = sp.tile([P, D], F32, tag="mct")
                nc.vector.tensor_scalar(
                    out=mct[:], in0=pm[:], scalar1=dinv_t[:, b:b + 1],
                    scalar2=None, op0=mybir.AluOpType.mult)
                nc.vector.tensor_tensor(
                    out=mct[:], in0=acc_m[:, b * D:(b + 1) * D], in1=mct[:],
                    op=mybir.AluOpType.add)
                nc.scalar.mul(mct[:], mct[:], 1.0 / (LAYERS + 1))
                nc.sync.dma_start(out=macc_in[b * P:(b + 1) * P, :], in_=mct[:])

            spmm_layer(m1bf_full, NBM, CHM, mmidx_t, mmlr_t, mmval_t, mm_l2_post)
            nc.gpsimd.collective_compute(
                "AllGather", mybir.AluOpType.bypass, replica_groups=RG,
                ins=[macc_in[:]], outs=[macc_full[:]])

            # ----------------- head -----------------------------------------
            with tc.tile_pool(name="hd", bufs=1) as hd, \
                 tc.tile_pool(name="hd2", bufs=1) as hd2, \
                 tc.tile_pool(name="hd_ps", bufs=2, space="PSUM") as hps, \
                 tc.tile_pool(name="hd_ps2", bufs=2, space="PSUM") as hps2:
                ident = hd.tile([P, P], F32)
                make_identity(nc, ident[:])
                ones_t = hd.tile([1, P], F32)
                nc.vector.memset(ones_t[:], 1.0)
                neg_t = hd.tile([1, P], F32)
                nc.vector.memset(neg_t[:], -1e9)
                w1m_t = hd.tile([P, 16], F32)
                nc.sync.dma_start(out=w1m_t[:], in_=aw1[0:D, :])
                w1s_t = hd.tile([P, 16], F32)
                nc.sync.dma_start(out=w1s_t[:], in_=aw1[D:2 * D, :])
                w2_t = hd.tile([16, 1], F32)
                nc.sync.dma_start(out=w2_t[:], in_=aw2[:])
                b1_t = hd.tile([16, 1], F32)
                nc.sync.dma_start(out=b1_t[:], in_=ab1.unsqueeze(1))
                b2_t = hd.tile([1, 1], F32)
                nc.sync.dma_start(out=b2_t[:], in_=ab2.unsqueeze(1))
                b2r_t = hd.tile([P, 1], F32)
                nc.gpsimd.partition_broadcast(b2r_t[:], b2_t[:])
                pw1_t = hd.tile([P, 3 * 8], F32)
                nc.sync.dma_start(
                    out=pw1_t[:].rearrange("p (c h) -> p c h", h=8),
                    in_=pw1.rearrange("(c p) h -> p c h", p=P))
                pb1_t = hd.tile([8, 1], F32)
                nc.sync.dma_start(out=pb1_t[:], in_=pb1.unsqueeze(1))
                pw2_t = hd.tile([8, 1], F32)
                nc.sync.dma_start(out=pw2_t[:], in_=pw2[:])
                pb2_t = hd.tile([1, 1], F32)
                nc.sync.dma_start(out=pb2_t[:], in_=pb2.unsqueeze(1))

                NLB = L * P          # (l, b) flat size per batch tile
                NCK = NLB // 512     # 512-wide chunks

                for t in range(BT):
                    midx_t = hd2.tile([P, L], I32, tag="midx")
                    nc.sync.dma_start(out=midx_t[:],
                                      in_=memb_idx[t * P:(t + 1) * P, :])
                    me_t = hd2.tile([P, L * D], F32, tag="me")
                    nc.gpsimd.indirect_dma_start(
                        out=me_t[:], out_offset=None, in_=uif_full[:],
                        in_offset=bass.IndirectOffsetOnAxis(ap=midx_t[:], axis=0))
                    et_t = hd2.tile([P, L * D], F32, tag="et")
                    for l in range(L):
                        ptr = hps.tile([P, P], F32, tag="ptr")
                        nc.tensor.transpose(out=ptr[:],
                                            in_=me_t[:, l * D:(l + 1) * D],
                                            identity=ident[:])
                        nc.vector.tensor_copy(out=et_t[:, l * P:(l + 1) * P],
                                              in_=ptr[:])
                    sidx_t = hd2.tile([P, 1], I32, tag="sidx")
                    nc.sync.dma_start(out=sidx_t[:],
                                      in_=svc_idx[t * P:(t + 1) * P, :])
                    sv_t = hd2.tile([P, D], F32, tag="sv")
                    nc.gpsimd.indirect_dma_start(
                        out=sv_t[:], out_offset=None, in_=uif_full[:],
                        in_offset=bass.IndirectOffsetOnAxis(ap=sidx_t[:], axis=0),
                        element_offset=NU * D)
                    ptr = hps.tile([P, P], F32, tag="ptr")
                    nc.tensor.transpose(out=ptr[:], in_=sv_t[:], identity=ident[:])
                    svcT_t = hd2.tile([P, P], F32, tag="svcT")
                    nc.vector.tensor_copy(out=svcT_t[:], in_=ptr[:])
                    xidx_t = hd2.tile([P, 1], I32, tag="xidx")
                    nc.sync.dma_start(out=xidx_t[:],
                                      in_=mash_idx[t * P:(t + 1) * P, :])
                    ma_t = hd2.tile([P, D], F32, tag="ma")
                    nc.gpsimd.indirect_dma_start(
                        out=ma_t[:], out_offset=None, in_=macc_full[:],
                        in_offset=bass.IndirectOffsetOnAxis(ap=xidx_t[:], axis=0))
                    ptr = hps.tile([P, P], F32, tag="ptr")
                    nc.tensor.transpose(out=ptr[:], in_=ma_t[:], identity=ident[:])
                    maT_t = hd2.tile([P, P], F32, tag="maT")
                    nc.vector.tensor_copy(out=maT_t[:], in_=ptr[:])

                    # svc_term [16, b]
                    psv = hps2.tile([16, P], F32, tag="ps_small")
                    nc.tensor.matmul(out=psv[:], lhsT=w1s_t[:], rhs=svcT_t[:],
                                     start=True, stop=True)
                    svterm_t = hd2.tile([16, P], F32, tag="svterm")
                    nc.vector.tensor_copy(out=svterm_t[:], in_=psv[:])

                    # hidden [16, (l, b)]
                    hdn_t = hd2.tile([16, NLB], F32, tag="hdn")
                    lpc = 512 // P   # l's per 512 chunk
                    for n in range(NCK):
                        pmt = hps2.tile([16, 512], F32, tag="ps_small")
                        nc.tensor.matmul(out=pmt[:], lhsT=w1m_t[:],
                                         rhs=et_t[:, n * 512:(n + 1) * 512],
                                         start=True, stop=True)
                        tt = hd2.tile([16, 512], F32, tag="tt16")
                        nc.vector.tensor_tensor(
                            out=tt[:].rearrange("h (l b) -> h l b", b=P),
                            in0=pmt[:].rearrange("h (l b) -> h l b", b=P),
                            in1=svterm_t[:].unsqueeze(1).to_broadcast([16, lpc, P]),
                            op=mybir.AluOpType.add)
                        nc.scalar.activation(
                            out=hdn_t[:, n * 512:(n + 1) * 512], in_=tt[:],
                            func=mybir.ActivationFunctionType.Relu,
                            bias=b1_t[:], scale=1.0)

                    # scores [1, (l, b)]
                    sc_t = hd2.tile([1, NLB], F32, tag="sc")
                    for n in range(NCK):
                        pst = hps2.tile([1, 512], F32, tag="ps_small")
                        nc.tensor.matmul(out=pst[:], lhsT=w2_t[:],
                                         rhs=hdn_t[:, n * 512:(n + 1) * 512],
                                         start=True, stop=True)
                        nc.vector.tensor_copy(out=sc_t[:, n * 512:(n + 1) * 512],
                                              in_=pst[:])
                    mk_t = hd2.tile([1, NLB], F32, tag="mk")
                    nc.sync.dma_start(out=mk_t[:], in_=mask_lb[t:t + 1, :])

                    # ew = exp(score - 1e9*mask + b2) replicated to 128 parts
                    ew_t = hd2.tile([P, NLB], F32, tag="ew")
                    for n in range(NCK):
                        prt = hps.tile([P, 512], F32, tag="prt")
                        nc.tensor.matmul(out=prt[:], lhsT=ones_t[:],
                                         rhs=sc_t[:, n * 512:(n + 1) * 512],
                                         start=True, stop=False)
                        nc.tensor.matmul(out=prt[:], lhsT=neg_t[:],
                                         rhs=mk_t[:, n * 512:(n + 1) * 512],
                                         start=False, stop=True)
                        nc.scalar.activation(
                            out=ew_t[:, n * 512:(n + 1) * 512], in_=prt[:],
                            func=mybir.ActivationFunctionType.Exp,
                            bias=b2r_t[:], scale=1.0)

                    den_t = hd2.tile([P, P], F32, tag="den")
                    nc.vector.tensor_reduce(
                        out=den_t[:],
                        in_=ew_t[:].rearrange("p (l b) -> p b l", b=P),
                        axis=mybir.AxisListType.X, op=mybir.AluOpType.add)
                    rden_t = hd2.tile([P, P], F32, tag="rden")
                    nc.vector.reciprocal(rden_t[:], den_t[:])
                    # weighted member sum (in-place ew <- ET * ew)
                    nc.vector.tensor_tensor(out=ew_t[:], in0=et_t[:], in1=ew_t[:],
                                            op=mybir.AluOpType.mult)
                    gatt_t = hd2.tile([P, P], F32, tag="gatt")
                    nc.vector.tensor_reduce(
                        out=gatt_t[:],
                        in_=ew_t[:].rearrange("p (l b) -> p b l", b=P),
                        axis=mybir.AxisListType.X, op=mybir.AluOpType.add)
                    nc.vector.tensor_tensor(out=gatt_t[:], in0=gatt_t[:],
                                            in1=rden_t[:],
                                            op=mybir.AluOpType.mult)
                    # mashup_emb^T / elem^T
                    nc.vector.tensor_tensor(out=maT_t[:], in0=gatt_t[:],
                                            in1=maT_t[:], op=mybir.AluOpType.add)
                    elem_t = hd2.tile([P, P], F32, tag="elem")
                    nc.vector.tensor_tensor(out=elem_t[:], in0=maT_t[:],
                                            in1=svcT_t[:],
                                            op=mybir.AluOpType.mult)
                    # predict
                    ppd = hps2.tile([8, P], F32, tag="ps_small")
                    for c, rhs in enumerate((elem_t, maT_t, svcT_t)):
                        nc.tensor.matmul(out=ppd[:], lhsT=pw1_t[:, c * 8:(c + 1) * 8],
                                         rhs=rhs[:], start=(c == 0), stop=(c == 2))
                    hp_t = hd2.tile([8, P], F32, tag="hp")
                    nc.scalar.activation(out=hp_t[:], in_=ppd[:],
                                         func=mybir.ActivationFunctionType.Relu,
                                         bias=pb1_t[:], scale=1.0)
                    pyt = hps2.tile([1, P], F32, tag="ps_small")
                    nc.tensor.matmul(out=pyt[:], lhsT=pw2_t[:], rhs=hp_t[:],
                                     start=True, stop=True)
                    y_t = hd2.tile([1, P], F32, tag="yt")
                    nc.scalar.activation(out=y_t[:], in_=pyt[:],
                                         func=mybir.ActivationFunctionType.Sigmoid,
                                         bias=pb2_t[:], scale=1.0)
                    nc.sync.dma_start(out=y[t * P:(t + 1) * P, :], in_=y_t[:])

    nc.compile()
    return nc


# ---------------------------------------------------------------------------
# host orchestration
# ---------------------------------------------------------------------------

def prepare(inputs, bf16=True, gg=2):
    NU, D = inputs['user_tbl'].shape
    NS = inputs['service_tbl'].shape[0]
    NM = inputs['mashup_tbl'].shape[0]
    B, L = inputs['member_masked'].shape
    edge_np = NP_BF16 if bf16 else np.float32

    NBU = _cdiv(NU + NS, N_CORES * P)
    NBM = _cdiv(NM, N_CORES * P)
    RWU, RWM = NBU * P, NBM * P
    BT = B // (N_CORES * P)

    CHU = max_chunks(inputs['adj_rows'], NBU)
    CHM = max_chunks(inputs['A_rows'], NBM)

    ui_idx, ui_lr, ui_val = pack_edges(
        np.asarray(inputs['adj_rows']), np.asarray(inputs['adj_cols']),
        np.asarray(inputs['adj_vals'], np.float32), NBU, CHU, edge_np)
    mm_idx, mm_lr, mm_val = pack_edges(
        np.asarray(inputs['A_rows']), np.asarray(inputs['A_cols']),
        np.asarray(inputs['A_vals'], np.float32), NBM, CHM, edge_np)

    x0 = np.zeros((N_CORES * RWU, D), np.float32)
    x0[:NU] = inputs['user_tbl']
    x0[NU:NU + NS] = inputs['service_tbl']
    m0 = np.zeros((N_CORES * RWM, D), np.float32)
    m0[:NM] = inputs['mashup_tbl']
    dv = np.zeros(N_CORES * RWM, np.float32)
    dv[:NM] = inputs['d_inv']
    dv = dv.reshape(N_CORES, NBM, P).transpose(0, 2, 1).copy()  # (c, p, b)

    BC = BT * P   # batch rows per core
    mask = np.asarray(inputs['mask'], np.float32).reshape(N_CORES, BT, P, L)
    mask_lb = mask.transpose(0, 1, 3, 2).reshape(N_CORES, BT, L * P).copy()

    cfg = dict(NU=NU, NS=NS, NM=NM, D=D, L=L, NBU=NBU, NBM=NBM,
               CHU=CHU, CHM=CHM, BT=BT, bf16=bf16, GG=gg)

    in_maps = []
    for k in range(N_CORES):
        in_maps.append({
            'x0s': x0[k * RWU:(k + 1) * RWU],
            'm0s': m0[k * RWM:(k + 1) * RWM],
            'ui_idx': ui_idx[k], 'ui_lr': ui_lr[k], 'ui_val': ui_val[k],
            'mm_idx': mm_idx[k], 'mm_lr': mm_lr[k], 'mm_val': mm_val[k],
            'dinv': dv[k],
            'memb_idx': np.asarray(inputs['member_masked'], np.int32)
                [k * BC:(k + 1) * BC],
            'svc_idx': np.asarray(inputs['service_inputs'], np.int32)
                [k * BC:(k + 1) * BC].reshape(BC, 1),
            'mash_idx': np.asarray(inputs['mashup_inputs'], np.int32)
                [k * BC:(k + 1) * BC].reshape(BC, 1),
            'mask_lb': mask_lb[k],
            'aw1': np.asarray(inputs['att_w1'], np.float32),
            'ab1': np.asarray(inputs['att_b1'], np.float32),
            'aw2': np.asarray(inputs['att_w2'], np.float32),
            'ab2': np.asarray(inputs['att_b2'], np.float32),
            'pw1': np.asarray(inputs['pred_w1'], np.float32),
            'pb1': np.asarray(inputs['pred_b1'], np.float32),
            'pw2': np.asarray(inputs['pred_w2'], np.float32),
            'pb2': np.asarray(inputs['pred_b2'], np.float32),
        })
    return cfg, in_maps


_CACHE = {}


def run(inputs, bf16=True, gg=2, trace=False):
    from concourse.bass_utils import run_bass_kernel_spmd
    cfg, in_maps = prepare(inputs, bf16=bf16, gg=gg)
    key = tuple(sorted((k, v) for k, v in cfg.items()))
    if key not in _CACHE:
        _CACHE[key] = build_program(cfg)
    nc = _CACHE[key]
    res = run_bass_kernel_spmd(nc, in_maps, core_ids=list(range(N_CORES)),
                               trace=False)
    yy = np.concatenate([r['y'] for r in res.results], axis=0)
    return yy, res


def make_timed_runner(nc, in_maps):
    """Build the sharded PJRT executable with inputs staged on device once.
    Returns (call_fn, out_names) where call_fn() runs one execution and
    blocks; outputs stay on device (only donated zero-buffers re-upload).
    """
    import jax
    from jax.sharding import Mesh, PartitionSpec, NamedSharding
    from jax.experimental.shard_map import shard_map
    from concourse import bass2jax
    from concourse.bass2jax import _bass_exec_p, install_neuronx_cc_hook
    import concourse.mybir as mb

    install_neuronx_cc_hook()
    nc_ = nc
    pname = nc_.partition_id_tensor.name if nc_.partition_id_tensor else None
    in_names, out_names, out_avals, zero_outs = [], [], [], []
    for alloc in nc_.m.functions[0].allocations:
        if not isinstance(alloc, mybir.MemoryLocationSet):
            continue
        name = alloc.memorylocations[0].name
        if alloc.kind == "ExternalInput":
            if name != pname:
                in_names.append(name)
        elif alloc.kind == "ExternalOutput":
            out_names.append(name)
            shape = tuple(alloc.tensor_shape)
            dtype = mybir.dt.np(alloc.dtype)
            out_avals.append(jax.core.ShapedArray(shape, dtype))
            zero_outs.append(np.zeros(shape, dtype))
    n_params = len(in_names)
    n_outs = len(out_avals)
    all_names = in_names + out_names
    if pname is not None:
        all_names = all_names + [pname]

    def _body(*args):
        operands = list(args)
        if pname is not None:
            operands.append(bass2jax.partition_id_tensor())
        outs = _bass_exec_p.bind(
            *operands, out_avals=tuple(out_avals), in_names=tuple(all_names),
            out_names=tuple(out_names), lowering_input_output_aliases=(),
            sim_require_finite=True, sim_require_nnan=True, nc=nc_)
        return tuple(outs)

    devices = jax.devices()[:N_CORES]
    mesh = Mesh(np.asarray(devices), ("core",))
    in_specs = (PartitionSpec("core"),) * (n_params + n_outs)
    out_specs = (PartitionSpec("core"),) * n_outs
    donate = tuple(range(n_params, n_params + n_outs))
    sharded = jax.jit(
        shard_map(_body, mesh=mesh, in_specs=in_specs, out_specs=out_specs,
                  check_rep=False),
        donate_argnums=donate, keep_unused=True)
    sh = NamedSharding(mesh, PartitionSpec("core"))
    dev_in = [
        jax.device_put(
            np.concatenate([np.asarray(in_maps[c][nm]) for c in range(N_CORES)],
                           axis=0), sh)
        for nm in in_names]

    def call():
        zeros = [
            jax.device_put(np.zeros((N_CORES * z.shape[0], *z.shape[1:]), z.dtype), sh)
            for z in zero_outs]
        outs = sharded(*dev_in, *zeros)
        jax.block_until_ready(outs)
        return outs

    return call, out_names


def kernel(**inputs) -> np.ndarray:
    yy, _ = run(inputs, bf16=True)
    return yy.astype(np.float32)

